# revision 2
# baseline (speedup 1.0000x reference)
"""Trainium2 Bass kernel for nn_CapsNet_69114613730132.

Strategy (8 NeuronCores, SPMD, one program + per-core input data):
  * conv1 (3->128, k9, 24x24): replicated as a 243-deep matmul, but each
    core only computes the 176 output columns its conv2 shard consumes
    (host reorders/duplicates im2col columns per core).
  * conv2 / PrimaryCaps (128->256, k9, s2): contraction (ic, dy, dx) is
    sharded by (dy,dx) kernel position: 81 positions padded to 88 = 8*11
    slots; each core accumulates 11 slots x 2 oc2-halves of matmuls into
    a (256,16) partial.
  * One AllGather (floor ~4.6us, vs ~9.7us AllReduce) exchanges the 16KB
    partials; every core sums the 8 partials locally on VectorE.
  * squash + DigitCaps routing: the routing loop is degenerate (cij is
    constant 1/512), so the whole thing collapses to a 4096->160 matvec
    of W against squash(p), followed by an elementwise squash. Sharded by
    the 160-dim output (20 per core) => no second collective; the host
    just concatenates the 8 (1,20) outputs.
  * All PE compute in bf16 (weights host-cast), f32 PSUM/vector math.

kernel(**inputs) takes the FULL unsharded inputs and returns the full
(1,1,10,16,1) float32 output.
"""
import numpy as np
import ml_dtypes

import concourse.bass as bass
import concourse.bacc as bacc
import concourse.tile as tile
import concourse.mybir as mybir
from concourse.bass_utils import run_bass_kernel_spmd

BF16 = ml_dtypes.bfloat16
F32 = mybir.dt.float32
BF = mybir.dt.bfloat16

NCORES = 8
SLOTS = 11          # (dy,dx) slots per core; 81 real + 7 zero-padded
KI = 20             # digitcaps output elems per core (160 = 8*20)
NPOS = SLOTS * 16   # conv1 columns materialized per core


# --------------------------------------------------------------------------
# Host-side input marshalling (pure layout transforms + dtype casts)
# --------------------------------------------------------------------------

def _host_prep(x, conv_w, conv_b, pri_w, pri_b, W):
    x = np.asarray(x, np.float32)
    conv_w = np.asarray(conv_w, np.float32)
    conv_b = np.asarray(conv_b, np.float32)
    pri_w = np.asarray(pri_w, np.float32)
    pri_b = np.asarray(pri_b, np.float32)
    W = np.asarray(W, np.float32)

    # base im2col of x: (243, 256), row (c,dy,dx), col (oy*16+ox)
    im2col1 = np.empty((3, 9, 9, 16, 16), np.float32)
    for dy in range(9):
        for dx in range(9):
            im2col1[:, dy, dx] = x[0, :, dy:dy + 16, dx:dx + 16]
    im2col1 = im2col1.reshape(243, 256)

    W1T = conv_w.reshape(128, 243).T.copy()  # (243, 128)

    # (dydx, ic, oc2) with oc2 = cap*8 + j; padded to 88 slots
    W2re = pri_w.reshape(256, 128, 9, 9).transpose(2, 3, 1, 0).reshape(81, 128, 256)
    W2pad = np.zeros((NCORES * SLOTS, 128, 256), np.float32)
    W2pad[:81] = W2re

    # conv2-output-position -> conv1-output-column table, per slot
    postab = np.zeros((NCORES * SLOTS, 16), np.int64)
    for slot in range(81):
        dy, dx = divmod(slot, 9)
        for oy in range(4):
            for ox in range(4):
                postab[slot, oy * 4 + ox] = (2 * oy + dy) * 16 + (2 * ox + dx)

    # digitcaps weights V[h, s, p, ki]:
    #   oc2 = 128h+p; cap=oc2>>3; j=oc2&7; n = cap*16 + j*2 + (s>>3); jj = s&7
    Wd = W[0]  # (512, 10, 16, 8)
    oc2 = np.arange(256)
    n_base = (oc2 >> 3) * 16 + (oc2 & 7) * 2  # (256,)
    V = np.empty((2, 16, 128, 160), np.float32)
    for s in range(16):
        sel = Wd[n_base + (s >> 3), :, :, s & 7]      # (256, 10, 16)
        V[:, s] = sel.reshape(2, 128, 160)

    pb2 = pri_b.reshape(2, 128).T.copy()  # (128, 2) [p, h]
    cb2 = conv_b.reshape(128, 1)

    per_core = []
    for c in range(NCORES):
        sl = slice(c * SLOTS, (c + 1) * SLOTS)
        cols = im2col1[:, postab[sl].reshape(-1)]                 # (243, 176)
        w2s = W2pad[sl].transpose(1, 0, 2).reshape(128, SLOTS * 256)
        vsl = V[:, :, :, c * KI:(c + 1) * KI]                     # (2,16,128,20)
        vsl = vsl.transpose(2, 0, 1, 3).reshape(128, 32 * KI)     # (128, 640)
        per_core.append({
            "im_a": np.ascontiguousarray(cols[:128]).astype(BF16),
            "im_b": np.ascontiguousarray(cols[128:]).astype(BF16),
            "w1t_a": np.ascontiguousarray(W1T[:128]).astype(BF16),
            "w1t_b": np.ascontiguousarray(W1T[128:]).astype(BF16),
            "w2s": w2s.astype(BF16),
            "v": np.ascontiguousarray(vsl).astype(BF16),
            "cb": cb2,
            "pb": pb2,
        })
    return per_core


INPUT_SPECS = {
    "im_a": ((128, NPOS), BF),
    "im_b": ((115, NPOS), BF),
    "w1t_a": ((128, 128), BF),
    "w1t_b": ((115, 128), BF),
    "w2s": ((128, SLOTS * 256), BF),
    "v": ((128, 32 * KI), BF),
    "cb": ((128, 1), F32),
    "pb": ((128, 2), F32),
}


# --------------------------------------------------------------------------
# Device IR
# --------------------------------------------------------------------------

def emit_kernel(tc, out_ap, ins):
    nc = tc.nc
    with (
        tc.tile_pool(name="sb", bufs=1) as sb,
        tc.tile_pool(name="ps", bufs=1, space="PSUM") as ps,
        tc.tile_pool(name="dram", bufs=1, space="DRAM") as dram,
    ):
        # ---- load inputs to SBUF
        im_a_sb = sb.tile([128, NPOS], BF)
        im_b_sb = sb.tile([115, NPOS], BF)
        w1t_a_sb = sb.tile([128, 128], BF)
        w1t_b_sb = sb.tile([115, 128], BF)
        w2s_sb = sb.tile([128, SLOTS * 256], BF)
        v_sb = sb.tile([128, 32 * KI], BF)
        cb_sb = sb.tile([128, 1], F32)
        pb_sb = sb.tile([128, 2], F32)
        for t, name in (
            (im_a_sb, "im_a"), (im_b_sb, "im_b"),
            (w1t_a_sb, "w1t_a"), (w1t_b_sb, "w1t_b"),
            (w2s_sb, "w2s"), (v_sb, "v"), (cb_sb, "cb"), (pb_sb, "pb"),
        ):
            nc.sync.dma_start(t[:], ins[name][:])

        # ---- conv1: h = W1T.T @ im2col_c + conv_b  -> (128, NPOS) bf16
        psum1 = ps.tile([128, NPOS], F32)
        nc.tensor.matmul(psum1[:], w1t_a_sb[:], im_a_sb[:], start=True, stop=False)
        nc.tensor.matmul(psum1[:], w1t_b_sb[:], im_b_sb[:], start=False, stop=True)
        h_sb = sb.tile([128, NPOS], BF)
        nc.vector.tensor_scalar_add(h_sb[:], psum1[:], cb_sb[:])

        # ---- conv2 partial: two PSUM banks, [p, s] per oc2-half
        psum2a = ps.tile([128, 16], F32)
        psum2b = ps.tile([128, 16], F32)
        halves = (psum2a, psum2b)
        for i in range(SLOTS):
            for hh in range(2):
                nc.tensor.matmul(
                    halves[hh][:],
                    w2s_sb[:, i * 256 + hh * 128: i * 256 + (hh + 1) * 128],
                    h_sb[:, i * 16:(i + 1) * 16],
                    start=(i == 0), stop=(i == SLOTS - 1),
                )
        part_sb = sb.tile([128, 32], F32)
        nc.vector.tensor_copy(part_sb[:, 0:16], psum2a[:])
        nc.vector.tensor_copy(part_sb[:, 16:32], psum2b[:])

        # ---- AllGather the (256,16) partials
        bounce_in = dram.tile([256, 16], F32)
        bounce_out = dram.tile([NCORES * 256, 16], F32, addr_space="Shared")
        nc.sync.dma_start(
            bounce_in.rearrange("(h p) s -> p h s", h=2),
            part_sb[:].rearrange("p (h s) -> p h s", h=2),
        )
        nc.gpsimd.collective_compute(
            "AllGather",
            mybir.AluOpType.bypass,
            replica_groups=[list(range(NCORES))],
            ins=[bounce_in.opt()],
            outs=[bounce_out.opt()],
        )
        # gathered rows: 256r + 128h + p = (2r+h)*128 + p -> [p, a*16+s], a=2r+h
        g_sb = sb.tile([128, 256], F32)
        nc.sync.dma_start(
            g_sb[:].rearrange("p (a s) -> p a s", a=16),
            bounce_out.rearrange("(a p) s -> p a s", p=128),
        )

        # ---- sum over ranks + pri_b bias -> x2 (128, 32) [p, h*16+s]
        x2 = sb.tile([128, 32], F32)
        g4 = g_sb[:].rearrange("p (r h s) -> p h s r", r=NCORES, h=2)
        x2v = x2[:].rearrange("p (h s) -> p h s", h=2)
        x2b = sb.tile([128, 32], F32)
        for hh in range(2):
            nc.vector.tensor_reduce(
                x2v[:, hh], g4[:, hh], axis=mybir.AxisListType.X,
                op=mybir.AluOpType.add,
            )
            nc.vector.tensor_scalar_add(
                x2b[:, hh * 16:(hh + 1) * 16],
                x2[:, hh * 16:(hh + 1) * 16],
                pb_sb[:, hh:hh + 1],
            )

        # ---- squash factors per (p, h, s_hi) group of 8
        # f = sqrt(sq)/512 / (1+sq)   (1/512 cij folded in)
        t2 = sb.tile([128, 32], F32)
        nc.vector.tensor_mul(t2[:], x2b[:], x2b[:])
        sq = sb.tile([128, 4], F32)
        nc.vector.tensor_reduce(
            sq[:], t2[:].rearrange("p (g e) -> p g e", e=8),
            axis=mybir.AxisListType.X, op=mybir.AluOpType.add,
        )
        r_ = sb.tile([128, 4], F32)
        nc.scalar.activation(
            r_[:], sq[:], mybir.ActivationFunctionType.Sqrt,
            scale=1.0 / (512.0 * 512.0),
        )
        d2 = sb.tile([128, 4], F32)
        nc.vector.tensor_scalar_add(d2[:], sq[:], 1.0)
        rec2 = sb.tile([128, 4], F32)
        nc.vector.reciprocal(rec2[:], d2[:])
        f_ = sb.tile([128, 4], F32)
        nc.vector.tensor_mul(f_[:], r_[:], rec2[:])

        u_sb = sb.tile([128, 32], BF)
        for g in range(4):
            nc.vector.tensor_scalar_mul(
                u_sb[:, g * 8:(g + 1) * 8],
                x2b[:, g * 8:(g + 1) * 8],
                f_[:, g:g + 1],
            )

        # ---- digitcaps matvec: psum_d[0, ki] = sum_{h,s,p} u * V
        psum_d = ps.tile([1, KI], F32)
        for idx in range(32):
            nc.tensor.matmul(
                psum_d[:],
                u_sb[:, idx:idx + 1],
                v_sb[:, idx * KI:(idx + 1) * KI],
                start=(idx == 0), stop=(idx == 31),
            )

        # ---- final elementwise squash: vij = s*|s|/(1+s^2)
        s_sb = sb.tile([1, KI], F32)
        nc.vector.tensor_copy(s_sb[:], psum_d[:])
        t3 = sb.tile([1, KI], F32)
        nc.vector.tensor_mul(t3[:], s_sb[:], s_sb[:])
        d3 = sb.tile([1, KI], F32)
        nc.vector.tensor_scalar_add(d3[:], t3[:], 1.0)
        rec3 = sb.tile([1, KI], F32)
        nc.vector.reciprocal(rec3[:], d3[:])
        a3 = sb.tile([1, KI], F32)
        nc.scalar.activation(a3[:], t3[:], mybir.ActivationFunctionType.Sqrt)
        m3 = sb.tile([1, KI], F32)
        nc.vector.tensor_mul(m3[:], a3[:], s_sb[:])
        o3 = sb.tile([1, KI], F32)
        nc.vector.tensor_mul(o3[:], m3[:], rec3[:])
        nc.sync.dma_start(out_ap[:], o3[:])


# --------------------------------------------------------------------------
# Build + run
# --------------------------------------------------------------------------

_CACHE = {}


def build_nc():
    nc = bacc.Bacc(
        "TRN2", target_bir_lowering=False, debug=False, num_devices=NCORES
    )
    ins = {
        name: nc.dram_tensor(name, list(shape), dt, kind="ExternalInput").ap()
        for name, (shape, dt) in INPUT_SPECS.items()
    }
    out_ap = nc.dram_tensor("out", [1, KI], F32, kind="ExternalOutput").ap()
    with tile.TileContext(nc) as tc:
        emit_kernel(tc, out_ap, ins)
    nc.compile()
    return nc


def kernel(**inputs):
    per_core = _host_prep(**inputs)
    if "nc" not in _CACHE:
        _CACHE["nc"] = build_nc()
    res = run_bass_kernel_spmd(
        _CACHE["nc"], per_core, core_ids=list(range(NCORES))
    )
    out = np.concatenate(
        [np.asarray(res.results[c]["out"], np.float32).reshape(-1)
         for c in range(NCORES)]
    )
    return out.reshape(1, 1, 10, 16, 1)


# revision 3
# speedup vs baseline: 2.5490x; 2.5490x over previous
"""Trainium2 Bass kernel for nn_CapsNet_69114613730132.

Strategy (8 NeuronCores, SPMD, zero collectives):
  The CapsNet routing loop is degenerate (self.bij is never updated, so
  cij stays 1/512) and collapses to: conv1 -> conv2 -> squash ->
  4096->160 matvec -> elementwise squash. The convolutions are tiny, so
  cross-core collectives (AllGather floor + a ~40us rank-alignment
  barrier measured on this fabric) cost more than replicating them.

  * Every core computes conv1 + conv2 (PrimaryCaps) + squash redundantly:
      conv1 as a 243-contraction matmul over a host-built im2col of x;
      conv2 as 81 (dy,dx) PSUM-accumulated matmuls over strided views of
      h (no im2col materialization), weights stationary, bf16.
  * The DigitCaps matvec output (160 = 10*16) is sharded 20-per-core via
    per-core weight slices => cores are fully independent; the host just
    concatenates the 8 (1,20) results. No communication at all.
  * All PE compute in bf16 (weights host-cast), f32 PSUM/vector math.

kernel(**inputs) takes the FULL unsharded inputs and returns the full
(1,1,10,16,1) float32 output.
"""
import numpy as np
import ml_dtypes

import concourse.bass as bass
import concourse.bacc as bacc
import concourse.tile as tile
import concourse.mybir as mybir
from concourse.bass_utils import run_bass_kernel_spmd

BF16 = ml_dtypes.bfloat16
F32 = mybir.dt.float32
BF = mybir.dt.bfloat16

NCORES = 8
KI = 20             # digitcaps output elems per core (160 = 8*20)
W2CHUNK = 9         # dydx positions per w2 DMA chunk (81 = 9*9)


# --------------------------------------------------------------------------
# Host-side input marshalling (pure layout transforms + dtype casts)
# --------------------------------------------------------------------------

def _host_prep(x, conv_w, conv_b, pri_w, pri_b, W):
    x = np.asarray(x, np.float32)
    conv_w = np.asarray(conv_w, np.float32)
    conv_b = np.asarray(conv_b, np.float32)
    pri_w = np.asarray(pri_w, np.float32)
    pri_b = np.asarray(pri_b, np.float32)
    W = np.asarray(W, np.float32)

    # im2col of x: (243, 256), row (c,dy,dx), col (oy*16+ox)
    im2col1 = np.empty((3, 9, 9, 16, 16), np.float32)
    for dy in range(9):
        for dx in range(9):
            im2col1[:, dy, dx] = x[0, :, dy:dy + 16, dx:dx + 16]
    im2col1 = im2col1.reshape(243, 256).astype(BF16)

    W1T = conv_w.reshape(128, 243).T.astype(BF16)  # (243, 128)

    # (ic, dydx*256 + oc2) with oc2 = cap*8 + j
    w2s = (pri_w.reshape(256, 128, 9, 9)
           .transpose(2, 3, 1, 0)          # (dy, dx, ic, oc2)
           .reshape(81, 128, 256)
           .transpose(1, 0, 2)             # (ic, dydx, oc2)
           .reshape(128, 81 * 256).astype(BF16))

    # digitcaps weights V[h, s, p, ki]:
    #   oc2 = 128h+p; cap=oc2>>3; j=oc2&7; n = cap*16 + j*2 + (s>>3); jj = s&7
    Wd = W[0]  # (512, 10, 16, 8)
    oc2 = np.arange(256)
    n_base = (oc2 >> 3) * 16 + (oc2 & 7) * 2
    V = np.empty((2, 16, 128, 160), np.float32)
    for s in range(16):
        sel = Wd[n_base + (s >> 3), :, :, s & 7]      # (256, 10, 16)
        V[:, s] = sel.reshape(2, 128, 160)

    pb2 = pri_b.reshape(2, 128).T.copy()  # (128, 2) [p, h]
    cb2 = conv_b.reshape(128, 1)

    shared = {
        "im_a": np.ascontiguousarray(im2col1[:128]),
        "im_b": np.ascontiguousarray(im2col1[128:]),
        "w1t_a": np.ascontiguousarray(W1T[:128]),
        "w1t_b": np.ascontiguousarray(W1T[128:]),
        "w2s": w2s,
        "cb": cb2,
        "pb": pb2,
    }
    per_core = []
    for c in range(NCORES):
        vsl = V[:, :, :, c * KI:(c + 1) * KI]                     # (2,16,128,20)
        vsl = vsl.transpose(2, 0, 1, 3).reshape(128, 32 * KI)     # (128, 640)
        d = dict(shared)
        d["v"] = np.ascontiguousarray(vsl).astype(BF16)
        per_core.append(d)
    return per_core


INPUT_SPECS = {
    "im_a": ((128, 256), BF),
    "im_b": ((115, 256), BF),
    "w1t_a": ((128, 128), BF),
    "w1t_b": ((115, 128), BF),
    "w2s": ((128, 81 * 256), BF),
    "v": ((128, 32 * KI), BF),
    "cb": ((128, 1), F32),
    "pb": ((128, 2), F32),
}


# --------------------------------------------------------------------------
# Device IR
# --------------------------------------------------------------------------

def emit_kernel(tc, out_ap, ins):
    nc = tc.nc
    nw2 = 81 // W2CHUNK
    with (
        tc.tile_pool(name="sb", bufs=1) as sb,
        tc.tile_pool(name="ps", bufs=1, space="PSUM") as ps,
    ):
        # ---- conv1 + small inputs on the sync HWDGE ring (FIFO order)
        im_a_sb = sb.tile([128, 256], BF)
        im_b_sb = sb.tile([115, 256], BF)
        w1t_a_sb = sb.tile([128, 128], BF)
        w1t_b_sb = sb.tile([115, 128], BF)
        for t, name in (
            (im_a_sb, "im_a"), (im_b_sb, "im_b"),
            (w1t_a_sb, "w1t_a"), (w1t_b_sb, "w1t_b"),
        ):
            nc.sync.dma_start(t[:], ins[name][:])
        # w2 chunks follow on the same FIFO ring: chunk j lands ~in order
        w2t = []
        for j in range(nw2):
            wt = sb.tile([128, W2CHUNK * 256], BF, name=f"w2t{j}")
            nc.sync.dma_start(
                wt[:], ins["w2s"][:, j * W2CHUNK * 256:(j + 1) * W2CHUNK * 256])
            w2t.append(wt)
        # small/late inputs on the scalar HWDGE ring (parallel to sync's)
        cb_sb = sb.tile([128, 1], F32)
        pb_sb = sb.tile([128, 2], F32)
        v_sb = sb.tile([128, 32 * KI], BF)
        nc.scalar.dma_start(cb_sb[:], ins["cb"][:])
        nc.scalar.dma_start(pb_sb[:], ins["pb"][:])
        nc.scalar.dma_start(v_sb[:], ins["v"][:])

        # ---- conv1: h = W1T.T @ im2col + conv_b  -> (128, 256) bf16
        psum1 = ps.tile([128, 256], F32)
        nc.tensor.matmul(psum1[:], w1t_a_sb[:], im_a_sb[:], start=True, stop=False)
        nc.tensor.matmul(psum1[:], w1t_b_sb[:], im_b_sb[:], start=False, stop=True)
        h_sb = sb.tile([128, 256], BF)
        nc.vector.tensor_scalar_add(h_sb[:], psum1[:], cb_sb[:])
        h4 = h_sb[:].rearrange("p (y x) -> p y x", y=16)

        # ---- conv2: 81 strided-view matmuls per oc2-half, PSUM-accumulated
        psum2a = ps.tile([128, 16], F32)
        psum2b = ps.tile([128, 16], F32)
        halves = (psum2a, psum2b)
        for dydx in range(81):
            dy, dx = divmod(dydx, 9)
            j, jj = divmod(dydx, W2CHUNK)
            rhs = h4[:, dy:dy + 8:2, dx:dx + 8:2]
            for hh in range(2):
                nc.tensor.matmul(
                    halves[hh][:],
                    w2t[j][:, jj * 256 + hh * 128: jj * 256 + (hh + 1) * 128],
                    rhs,
                    start=(dydx == 0), stop=(dydx == 80),
                )

        # ---- + pri_b -> x2b (128, 32) [p, h*16+s]
        x2b = sb.tile([128, 32], F32)
        for hh in range(2):
            nc.vector.tensor_scalar_add(
                x2b[:, hh * 16:(hh + 1) * 16], halves[hh][:], pb_sb[:, hh:hh + 1])

        # ---- squash factors per (p, h, s_hi) group of 8
        # f = sqrt(sq)/512 / (1+sq)   (1/512 cij folded in)
        t2 = sb.tile([128, 32], F32)
        nc.vector.tensor_mul(t2[:], x2b[:], x2b[:])
        sq = sb.tile([128, 4], F32)
        nc.vector.tensor_reduce(
            sq[:], t2[:].rearrange("p (g e) -> p g e", e=8),
            axis=mybir.AxisListType.X, op=mybir.AluOpType.add,
        )
        r_ = sb.tile([128, 4], F32)
        nc.scalar.activation(
            r_[:], sq[:], mybir.ActivationFunctionType.Sqrt,
            scale=1.0 / (512.0 * 512.0),
        )
        d2 = sb.tile([128, 4], F32)
        nc.vector.tensor_scalar_add(d2[:], sq[:], 1.0)
        rec2 = sb.tile([128, 4], F32)
        nc.vector.reciprocal(rec2[:], d2[:])
        f_ = sb.tile([128, 4], F32)
        nc.vector.tensor_mul(f_[:], r_[:], rec2[:])

        u_sb = sb.tile([128, 32], BF)
        for g in range(4):
            nc.vector.tensor_scalar_mul(
                u_sb[:, g * 8:(g + 1) * 8],
                x2b[:, g * 8:(g + 1) * 8],
                f_[:, g:g + 1],
            )

        # ---- digitcaps matvec: psum_d[0, ki] = sum_{h,s,p} u * V
        psum_d = ps.tile([1, KI], F32)
        for idx in range(32):
            nc.tensor.matmul(
                psum_d[:],
                u_sb[:, idx:idx + 1],
                v_sb[:, idx * KI:(idx + 1) * KI],
                start=(idx == 0), stop=(idx == 31),
            )

        # ---- final elementwise squash: vij = s*|s|/(1+s^2)
        s_sb = sb.tile([1, KI], F32)
        nc.vector.tensor_copy(s_sb[:], psum_d[:])
        t3 = sb.tile([1, KI], F32)
        nc.vector.tensor_mul(t3[:], s_sb[:], s_sb[:])
        d3 = sb.tile([1, KI], F32)
        nc.vector.tensor_scalar_add(d3[:], t3[:], 1.0)
        rec3 = sb.tile([1, KI], F32)
        nc.vector.reciprocal(rec3[:], d3[:])
        a3 = sb.tile([1, KI], F32)
        nc.scalar.activation(a3[:], t3[:], mybir.ActivationFunctionType.Sqrt)
        m3 = sb.tile([1, KI], F32)
        nc.vector.tensor_mul(m3[:], a3[:], s_sb[:])
        o3 = sb.tile([1, KI], F32)
        nc.vector.tensor_mul(o3[:], m3[:], rec3[:])
        nc.sync.dma_start(out_ap[:], o3[:])


# --------------------------------------------------------------------------
# Build + run
# --------------------------------------------------------------------------

_CACHE = {}


def build_nc():
    nc = bacc.Bacc(
        "TRN2", target_bir_lowering=False, debug=False, num_devices=NCORES
    )
    ins = {
        name: nc.dram_tensor(name, list(shape), dt, kind="ExternalInput").ap()
        for name, (shape, dt) in INPUT_SPECS.items()
    }
    out_ap = nc.dram_tensor("out", [1, KI], F32, kind="ExternalOutput").ap()
    with tile.TileContext(nc) as tc:
        emit_kernel(tc, out_ap, ins)
    nc.compile()
    return nc


def kernel(**inputs):
    per_core = _host_prep(**inputs)
    if "nc" not in _CACHE:
        _CACHE["nc"] = build_nc()
    res = run_bass_kernel_spmd(
        _CACHE["nc"], per_core, core_ids=list(range(NCORES))
    )
    out = np.concatenate(
        [np.asarray(res.results[c]["out"], np.float32).reshape(-1)
         for c in range(NCORES)]
    )
    return out.reshape(1, 1, 10, 16, 1)
